# revision 44
# baseline (speedup 1.0000x reference)
"""Depth-weighted 3x3 conv (DepthConv) Trainium2 Bass kernel (V1, fp16).

Math (per batch element):
  sim[k, p] = exp(-|depth[p + off_k] - depth[p]|)   (9 taps, off = dh*W + dw)
  out[o, p] = sum_{c,k} W[o,c,k] * sim[k,p] * x[c, p + off_k] + bias[o]

Sharding: data-parallel over batch, one batch element per NeuronCore (8).

Per-core layout ("half-image stacking"):
  SBUF partitions = 64 channels x {top half-image, bottom half-image}.
  Free dim = flat padded image: 84 rows x 162 cols (guard row + 82 buffer
  rows + guard row; each row = [pad, 160, pad]).  Output pixel (local row
  j, col w) has center flat index q=(j+1)*162+(w+1) in both halves; tap
  (dh, dw) reads q + dh*162 + dw.

Tap symmetry: sim_{-m}[q] = sim_{+m}[q - off_m], so only 4 similarity maps
exist (center tap's sim == 1).  Per map m:
  tap +m uses xm_m[q]   = x[q+off]*map_m[q]   (x via parity-aligned copy)
  tap -m uses prod_m[q] = x[q]*map_m[q], read by the conv at q - off_m
  (matmul rhs reads have no alignment constraint).

Similarity: depth diffs computed COMPACTLY on 8 partitions (row 2m = map m
top half, 2m+1 = bottom), |.| on ACT, broadcast to the 64-channel
partitions either by PE selection-matmul into PSUM + ACT exp-applying
evacuation (PE_MAPS) or by compact exp + DMA partition-replication
(DMA_MAPS) -- split to balance PE/ACT/DMA.

Conv: per 810-px chunk, 9 taps x 2 halves fp16 K=64 matmuls accumulate
into ONE [128, 810] PSUM tile; each (half, bank) range is an independent
accumulation group (has_written clear is per partition -- HW-verified), so
one full-width ACT pass evacuates both halves (+bias, ->fp16).  Output is
DMA'd with SWDGE fp16->fp32 cast.
"""

import functools
import os
import sys

import numpy as np

for _p in ("/opt/trn_rl_repo",):
    if os.path.isdir(_p) and _p not in sys.path:
        sys.path.insert(0, _p)

import concourse.bass as bass
import concourse.mybir as mybir
import concourse.tile as tile
from concourse import bacc
from concourse.bass_utils import run_bass_kernel_spmd

# ---------------------------------------------------------------- constants
B, C, H, W = 8, 64, 160, 160
O = 64
KK = 9
WB = W + 2                 # 162 padded row width
HB = H // 2 + 2            # 82 buffer rows per half
GUARD = WB                 # one padded row of guard cells each side
FLAT = HB * WB             # 13284
FLATG = FLAT + 2 * GUARD   # 13608 (84 rows)
NROWG = FLATG // WB        # 84
Q0 = WB
Q1 = FLAT - WB             # span 12960 = 80 out-rows per half
NCORES = 8

NSEG = 8
SEGROWS = 80 // NSEG       # out-rows per segment (10)
SEGQ = SEGROWS * WB        # 1620
HALO = 164                 # even, >= max |off| (163)
WIN = SEGQ + 2 * HALO      # 1948
NCH = 2
CHW = SEGQ // NCH          # 810
SUBS = (512, 298)          # matmul N splits at the fp32 PSUM bank boundary
NCW = 4                    # compact-sub windows (decoupled from NSEG)
DROWS = FLATG // WB // NCW   # 21 guarded rows per compact-sub window
DWIN = DROWS * WB          # 3402

MAPS = [(0, 1, 1), (1, 0, WB), (1, 1, WB + 1), (1, -1, WB - 1)]
PE_MAPS = ()
DMA_MAPS = (0, 1, 2, 3)

F32 = mybir.dt.float32
F16 = mybir.dt.float16


def _tapidx(dh, dw):
    return (dh + 1) * 3 + (dw + 1)


def _lift(ap):
    return bass.AP(
        tensor=ap.tensor, offset=ap.offset, ap=[[0, 1]] + list(ap.ap)
    )


def _build_program():
    # Bacc: finalize() runs generate_event_semaphores (splits multi-sem
    # waits to the 1-wait-per-instruction hardware limit).
    nc = bacc.Bacc(None)
    x_d = nc.declare_dram_parameter("x", [C, H, W], F32, isOutput=False)
    d_d = nc.declare_dram_parameter("depth", [H, W], F32, isOutput=False)
    wt_d = nc.declare_dram_parameter("wt", [C, KK, O], F32, isOutput=False)
    sel_d = (
        nc.declare_dram_parameter(
            "selm", [8, len(PE_MAPS), 128], F32, isOutput=False
        )
        if PE_MAPS
        else None
    )
    b_d = nc.declare_dram_parameter("bias2", [2 * O], F32, isOutput=False)
    out_d = nc.declare_dram_parameter("out", [O, H, W], F16, isOutput=True)

    Exp = mybir.ActivationFunctionType.Exp
    Abs = mybir.ActivationFunctionType.Abs
    Ident = mybir.ActivationFunctionType.Identity

    with tile.TileContext(nc) as tc:
        with (
            tc.tile_pool(name="dramp", bufs=1, space="DRAM") as dramp,
            tc.tile_pool(name="singles", bufs=1) as singles,
            tc.tile_pool(name="d08p", bufs=2) as d08p,
            tc.tile_pool(name="simp", bufs=10) as simp,
            tc.tile_pool(name="prodp", bufs=9) as prodp,
            tc.tile_pool(name="xmp", bufs=6) as xmp,
            tc.tile_pool(name="stgp", bufs=3) as stgp,
            tc.tile_pool(name="cpsum", bufs=2, space="PSUM") as cpsum,
            tc.tile_pool(name="bpsum", bufs=2, space="PSUM") as bpsum,
        ):
            x2e = singles.tile([128, FLATG], F16)
            x2o = singles.tile([128, FLATG], F16)
            ts8 = singles.tile([8, FLATG], F16)
            wt = singles.tile([128, KK, O], F16)
            b2 = singles.tile([128, 1], F32)
            sel = (
                singles.tile([8, len(PE_MAPS), 128], F16)
                if PE_MAPS
                else None
            )

            # ---------------- loads (SWDGE fp32->fp16 casts)
            # zero only the pad/guard cells (a full-tensor memset costs
            # ~11us serial DVE time)
            x2vz = x2e.rearrange("p (r w) -> p r w", r=NROWG, w=WB)
            nc.vector.memset(x2e[0:64, 0 : 2 * WB], 0.0)
            nc.vector.memset(x2e[0:64, 83 * WB :], 0.0)
            nc.vector.memset(x2vz[0:64, 2:83, 0:1], 0.0)
            nc.vector.memset(x2vz[0:64, 2:83, 161:162], 0.0)
            nc.vector.memset(x2e[64:128, 0:WB], 0.0)
            nc.vector.memset(x2e[64:128, 82 * WB :], 0.0)
            nc.vector.memset(x2vz[64:128, 1:82, 0:1], 0.0)
            nc.vector.memset(x2vz[64:128, 1:82, 161:162], 0.0)
            nc.vector.memset(ts8[:, FLATG - 163 :], 0.0)
            # small zeros row for zeroing the DRAM depth pad
            zrow = singles.tile([1, 4096], F16)
            nc.vector.memset(zrow[:], 0.0)

            nc.gpsimd.dma_start(out=wt[0:64], in_=wt_d[:])
            nc.gpsimd.dma_start(out=wt[64:128], in_=wt_d[:])
            nc.sync.dma_start(
                out=b2[:], in_=b_d.rearrange("(p one) -> p one", one=1)
            )

            dpad = dramp.tile([2, FLATG], F16)
            zi = 0
            for h in range(2):
                done_z = 0
                while done_z < FLATG:
                    nz = min(4096, FLATG - done_z)
                    eng = nc.scalar if zi % 2 == 0 else nc.sync
                    eng.dma_start(
                        out=dpad[h : h + 1, done_z : done_z + nz],
                        in_=zrow[0:1, 0:nz],
                    )
                    zi += 1
                    done_z += nz
            dpv = dpad.rearrange("t (r w) -> t r w", r=NROWG, w=WB)
            nc.gpsimd.dma_start(
                out=dpv[0:1, 2:83, 1:161],
                in_=_lift(d_d[0:81, :]),
            )
            nc.gpsimd.dma_start(
                out=dpv[1:2, 1:82, 1:161],
                in_=_lift(d_d[79:160, :]),
            )
            dpad_f = dpad[:]
            # dp rows: ts8[m+4h, g] = D_h[g + off] -- contiguous reads
            for m, (dh, dw, off) in enumerate(MAPS):
                for h in range(2):
                    row = m + 4 * h
                    a = max(0, -off)
                    bnd = min(FLATG, FLATG - off)
                    src = bass.AP(
                        tensor=dpad_f.tensor,
                        offset=dpad_f.offset + h * FLATG + a + off,
                        ap=[[0, 1], [1, bnd - a]],
                    )
                    nc.scalar.dma_start(out=ts8[row : row + 1, a:bnd], in_=src)

            if sel is not None:
                nc.gpsimd.dma_start(out=sel[:], in_=sel_d[:])

            # contiguous fp32->fp16 x load into x2o's storage (staging),
            # then DVE scatter into the padded layout.  320-byte strided
            # DMA descriptors were the V1 prelude bottleneck.
            XCON = 81 * W  # 12960
            for r4 in range(0, 81, 21):
                rr = min(21, 81 - r4)
                nc.gpsimd.dma_start(
                    out=x2o[0:64, r4 * W : (r4 + rr) * W],
                    in_=x_d[:, r4 : r4 + rr, :].rearrange(
                        "c r w -> c (r w)"
                    ),
                )
                nc.gpsimd.dma_start(
                    out=x2o[64:128, r4 * W : (r4 + rr) * W],
                    in_=x_d[:, 79 + r4 : 79 + r4 + rr, :].rearrange(
                        "c r w -> c (r w)"
                    ),
                )
            x2v = x2e.rearrange("p (r w) -> p r w", r=NROWG, w=WB)
            stv = x2o[:, 0:XCON].rearrange("p (r w) -> p r w", r=81, w=W)
            for r4 in range(0, 81, 21):
                rr = min(21, 81 - r4)
                nc.vector.tensor_copy(
                    x2v[0:64, 2 + r4 : 2 + r4 + rr, 1:161],
                    stv[0:64, r4 : r4 + rr],
                )
                nc.vector.tensor_copy(
                    x2v[64:128, 1 + r4 : 1 + r4 + rr, 1:161],
                    stv[64:128, r4 : r4 + rr],
                )
            # odd-parity copy: x2o[:, j] = x2e[:, j+1], in 4 chunks
            # (overwrites the staging; WAR ordering handled by Tile)
            ch4 = (FLATG - 2) // 4 + 1
            for c4 in range(4):
                a4 = c4 * ch4
                b4 = min(FLATG - 2, a4 + ch4)
                nc.sync.dma_start(
                    out=x2o[:, a4:b4], in_=x2e[:, a4 + 1 : b4 + 1]
                )

            # padded fp16 depth in DRAM (bounce): contiguous flat reads for
            # the shifted dp rows and the replicated d0 rows.
            # ------------- compact sim, segmented: t -= d0, |t|, exp(-t)
            s8d = dramp.tile([8, FLATG], F16)
            for s in range(NCW):
                d08 = d08p.tile([8, DWIN], F16, tag="d08")
                w0 = DWIN * s
                for h in range(2):
                    src = bass.AP(
                        tensor=dpad_f.tensor,
                        offset=dpad_f.offset + h * FLATG + w0,
                        ap=[[0, 4], [1, DWIN]],
                    )
                    nc.scalar.dma_start(
                        out=d08[4 * h : 4 * h + 4, :], in_=src
                    )
                nc.vector.tensor_sub(
                    ts8[:, w0 : w0 + DWIN],
                    ts8[:, w0 : w0 + DWIN],
                    d08[:, :],
                )
                nc.scalar.activation(
                    out=ts8[:, w0 : w0 + DWIN],
                    in_=ts8[:, w0 : w0 + DWIN],
                    func=Abs,
                )
                nc.scalar.activation(
                    out=ts8[:, w0 : w0 + DWIN],
                    in_=ts8[:, w0 : w0 + DWIN],
                    func=Exp,
                    scale=-1.0,
                )
                nc.scalar.dma_start(
                    out=s8d[:, w0 : w0 + DWIN], in_=ts8[:, w0 : w0 + DWIN]
                )

            # ---------------- main loop
            for s in range(NSEG):
                qs = Q0 + s * SEGQ
                winbase = GUARD + qs - HALO       # even
                sims = []
                for m, (dh, dw, off) in enumerate(MAPS):
                    sim_m = simp.tile([128, WIN], F16, tag="sim")
                    sims.append(sim_m)
                    if m in PE_MAPS:
                        i = PE_MAPS.index(m)
                        done = 0
                        while done < WIN:
                            nb = min(1024, WIN - done)
                            ps = bpsum.tile([128, 1024], F32, tag="bps")
                            o2 = 0
                            while o2 < nb:
                                nn2 = min(512, nb - o2)
                                nc.tensor.matmul(
                                    ps[:, o2 : o2 + nn2],
                                    sel[:, i, :],
                                    ts8[
                                        :,
                                        winbase + done + o2 :
                                        winbase + done + o2 + nn2,
                                    ],
                                    start=True,
                                    stop=True,
                                )
                                o2 += nn2
                            nc.scalar.activation(
                                out=sim_m[:, done : done + nb],
                                in_=ps[:, 0:nb],
                                func=Ident,
                                scale=1.0,
                            )
                            done += nb
                    else:
                        for h in range(2):
                            row = m + 4 * h
                            base = s8d[:]
                            src = bass.AP(
                                tensor=base.tensor,
                                offset=base.offset + row * FLATG + winbase,
                                ap=[[0, 64], [1, WIN]],
                            )
                            nc.gpsimd.dma_start(
                                out=sim_m[64 * h : 64 * h + 64, :], in_=src
                            )

                prods = []
                for m in range(4):
                    pr = prodp.tile([128, WIN], F16, tag="prod")
                    prods.append(pr)
                    nc.vector.tensor_mul(
                        pr[:], x2e[:, winbase : winbase + WIN], sims[m][:]
                    )

                stg = stgp.tile([128, SEGROWS * W], F16, tag="stg")
                for j in range(NCH):
                    q = qs + j * CHW
                    so = j * CHW + HALO           # even
                    # width padded to 1024 (2 banks) so each partition's
                    # row is bank-aligned; only [:, :CHW] is used.
                    psum = cpsum.tile([128, 1024], F32, tag="cps")
                    xms = []
                    for m, (dh, dw, off) in enumerate(MAPS):
                        xm = xmp.tile([128, CHW], F16, tag="xm")
                        xms.append(xm)
                        if off % 2:
                            xsrc = x2o[
                                :,
                                GUARD + q + off - 1 :
                                GUARD + q + off - 1 + CHW,
                            ]
                        else:
                            xsrc = x2e[
                                :, GUARD + q + off : GUARD + q + off + CHW
                            ]
                        nc.vector.tensor_mul(
                            xm[:], xsrc, sims[m][:, so : so + CHW]
                        )
                    # taps as (wt-idx, rhs-fn); top/bot matmuls adjacent so
                    # the PE can run the two 64x64 quadrants concurrently.
                    o2 = 0
                    for si_, nn2 in enumerate(SUBS):
                        taps = []
                        for m, (dh, dw, off) in enumerate(MAPS):
                            po = so - off
                            taps.append(
                                (_tapidx(dh, dw), xms[m], o2)
                            )
                            taps.append(
                                (_tapidx(-dh, -dw), prods[m], po + o2)
                            )
                        taps.append((_tapidx(0, 0), x2e, GUARD + q + o2))
                        for ti, (widx, rsrc, roff) in enumerate(taps):
                            for half in range(2):
                                pl, ph = 64 * half, 64 * half + 64
                                nc.tensor.matmul(
                                    psum[pl:ph, o2 : o2 + nn2],
                                    wt[pl:ph, widx, :],
                                    rsrc[pl:ph, roff : roff + nn2],
                                    start=(ti == 0),
                                    stop=(ti == len(taps) - 1),
                                    skip_group_check=True,
                                )
                        o2 += nn2
                    # CHW = 810 = 5 padded rows; strip the pad columns in
                    # the evacuation (strided psum read, contiguous out)
                    pv = psum.rearrange("p (r w) -> p r w", r=1024 // 2, w=2)
                    nc.scalar.activation(
                        out=stg[
                            :, j * 5 * W : (j + 1) * 5 * W
                        ].rearrange("p (r w) -> p r w", r=5, w=W),
                        in_=bass.AP(
                            tensor=psum[:].tensor,
                            offset=psum[:].offset + 1,
                            ap=[list(psum[:].ap[0]), [WB, 5], [1, W]],
                        ),
                        func=Ident,
                        bias=b2[:],
                        scale=1.0,
                    )

                r0 = SEGROWS * s
                nc.gpsimd.dma_start(
                    out=out_d[:, r0 : r0 + SEGROWS, :].rearrange(
                        "c r w -> c (r w)"
                    ),
                    in_=stg[0:64, :],
                )
                nc.gpsimd.dma_start(
                    out=out_d[:, 80 + r0 : 80 + r0 + SEGROWS, :].rearrange(
                        "c r w -> c (r w)"
                    ),
                    in_=stg[64:128, :],
                )

    return nc


@functools.lru_cache(maxsize=1)
def _get_program():
    return _build_program()


def make_in_maps(x, depth, weights, bias):
    wt = np.ascontiguousarray(
        weights.reshape(O, C, KK).transpose(1, 2, 0)
    ).astype(np.float32)
    b2 = np.concatenate([bias, bias]).astype(np.float32)
    selm = np.zeros((8, max(1, len(PE_MAPS)), 128), np.float32)
    for i, m in enumerate(PE_MAPS):
        selm[m, i, 0:64] = 1.0
        selm[m + 4, i, 64:128] = 1.0
    base = {"wt": wt, "bias2": b2}
    if PE_MAPS:
        base["selm"] = selm
    return [
        {
            "x": np.ascontiguousarray(x[i]).astype(np.float32),
            "depth": np.ascontiguousarray(depth[i, 0]).astype(np.float32),
            **base,
        }
        for i in range(x.shape[0])
    ]


def kernel(x, depth, weights, bias):
    nc = _get_program()
    if not nc.is_finalized():
        nc.finalize()
    in_maps = make_in_maps(x, depth, weights, bias)
    res = run_bass_kernel_spmd(nc, in_maps, list(range(NCORES)))
    out = np.stack([np.asarray(res.results[i]["out"]) for i in range(NCORES)])
    return out.astype(np.float32)


# revision 45
# speedup vs baseline: 1.0153x; 1.0153x over previous
"""Depth-weighted 3x3 conv (DepthConv) Trainium2 Bass kernel (V1, fp16).

Math (per batch element):
  sim[k, p] = exp(-|depth[p + off_k] - depth[p]|)   (9 taps, off = dh*W + dw)
  out[o, p] = sum_{c,k} W[o,c,k] * sim[k,p] * x[c, p + off_k] + bias[o]

Sharding: data-parallel over batch, one batch element per NeuronCore (8).

Per-core layout ("half-image stacking"):
  SBUF partitions = 64 channels x {top half-image, bottom half-image}.
  Free dim = flat padded image: 84 rows x 162 cols (guard row + 82 buffer
  rows + guard row; each row = [pad, 160, pad]).  Output pixel (local row
  j, col w) has center flat index q=(j+1)*162+(w+1) in both halves; tap
  (dh, dw) reads q + dh*162 + dw.

Tap symmetry: sim_{-m}[q] = sim_{+m}[q - off_m], so only 4 similarity maps
exist (center tap's sim == 1).  Per map m:
  tap +m uses xm_m[q]   = x[q+off]*map_m[q]   (x via parity-aligned copy)
  tap -m uses prod_m[q] = x[q]*map_m[q], read by the conv at q - off_m
  (matmul rhs reads have no alignment constraint).

Similarity: depth diffs computed COMPACTLY on 8 partitions (row 2m = map m
top half, 2m+1 = bottom), |.| on ACT, broadcast to the 64-channel
partitions either by PE selection-matmul into PSUM + ACT exp-applying
evacuation (PE_MAPS) or by compact exp + DMA partition-replication
(DMA_MAPS) -- split to balance PE/ACT/DMA.

Conv: per 810-px chunk, 9 taps x 2 halves fp16 K=64 matmuls accumulate
into ONE [128, 810] PSUM tile; each (half, bank) range is an independent
accumulation group (has_written clear is per partition -- HW-verified), so
one full-width ACT pass evacuates both halves (+bias, ->fp16).  Output is
DMA'd with SWDGE fp16->fp32 cast.
"""

import functools
import os
import sys

import numpy as np

for _p in ("/opt/trn_rl_repo",):
    if os.path.isdir(_p) and _p not in sys.path:
        sys.path.insert(0, _p)

import concourse.bass as bass
import concourse.mybir as mybir
import concourse.tile as tile
from concourse import bacc
from concourse.bass_utils import run_bass_kernel_spmd

# ---------------------------------------------------------------- constants
B, C, H, W = 8, 64, 160, 160
O = 64
KK = 9
WB = W + 2                 # 162 padded row width
HB = H // 2 + 2            # 82 buffer rows per half
GUARD = WB                 # one padded row of guard cells each side
FLAT = HB * WB             # 13284
FLATG = FLAT + 2 * GUARD   # 13608 (84 rows)
NROWG = FLATG // WB        # 84
Q0 = WB
Q1 = FLAT - WB             # span 12960 = 80 out-rows per half
NCORES = 8

NSEG = 8
SEGROWS = 80 // NSEG       # out-rows per segment (10)
SEGQ = SEGROWS * WB        # 1620
HALO = 164                 # even, >= max |off| (163)
WIN = SEGQ + 2 * HALO      # 1948
NCH = 2
CHW = SEGQ // NCH          # 810
SUBS = (512, 298)          # matmul N splits at the fp32 PSUM bank boundary
NCW = 4                    # compact-sub windows (decoupled from NSEG)
DROWS = FLATG // WB // NCW   # 21 guarded rows per compact-sub window
DWIN = DROWS * WB          # 3402

MAPS = [(0, 1, 1), (1, 0, WB), (1, 1, WB + 1), (1, -1, WB - 1)]
PE_MAPS = ()
DMA_MAPS = (0, 1, 2, 3)

F32 = mybir.dt.float32
F16 = mybir.dt.float16


def _tapidx(dh, dw):
    return (dh + 1) * 3 + (dw + 1)


def _lift(ap):
    return bass.AP(
        tensor=ap.tensor, offset=ap.offset, ap=[[0, 1]] + list(ap.ap)
    )


def _build_program():
    # Bacc: finalize() runs generate_event_semaphores (splits multi-sem
    # waits to the 1-wait-per-instruction hardware limit).
    nc = bacc.Bacc(None)
    x_d = nc.declare_dram_parameter("x", [C, H, W], F32, isOutput=False)
    d_d = nc.declare_dram_parameter("depth", [H, W], F32, isOutput=False)
    wt_d = nc.declare_dram_parameter("wt", [C, KK, O], F32, isOutput=False)
    sel_d = (
        nc.declare_dram_parameter(
            "selm", [8, len(PE_MAPS), 128], F32, isOutput=False
        )
        if PE_MAPS
        else None
    )
    b_d = nc.declare_dram_parameter("bias2", [2 * O], F32, isOutput=False)
    out_d = nc.declare_dram_parameter("out", [O, H, W], F16, isOutput=True)

    Exp = mybir.ActivationFunctionType.Exp
    Abs = mybir.ActivationFunctionType.Abs
    Ident = mybir.ActivationFunctionType.Identity

    with tile.TileContext(nc) as tc:
        with (
            tc.tile_pool(name="dramp", bufs=1, space="DRAM") as dramp,
            tc.tile_pool(name="singles", bufs=1) as singles,
            tc.tile_pool(name="d08p", bufs=2) as d08p,
            tc.tile_pool(name="simp", bufs=10) as simp,
            tc.tile_pool(name="prodp", bufs=9) as prodp,
            tc.tile_pool(name="xmp", bufs=6) as xmp,
            tc.tile_pool(name="stgp", bufs=3) as stgp,
            tc.tile_pool(name="cpsum", bufs=2, space="PSUM") as cpsum,
            tc.tile_pool(name="bpsum", bufs=2, space="PSUM") as bpsum,
        ):
            x2e = singles.tile([128, FLATG], F16)
            x2o = singles.tile([128, FLATG], F16)
            ts8 = singles.tile([8, FLATG], F16)
            wt = singles.tile([128, KK, O], F16)
            b2 = singles.tile([128, 1], F32)
            sel = (
                singles.tile([8, len(PE_MAPS), 128], F16)
                if PE_MAPS
                else None
            )

            # ---------------- loads (SWDGE fp32->fp16 casts)
            # zero only the pad/guard cells (a full-tensor memset costs
            # ~11us serial DVE time)
            x2vz = x2e.rearrange("p (r w) -> p r w", r=NROWG, w=WB)
            nc.vector.memset(x2e[0:64, 0 : 2 * WB], 0.0)
            nc.vector.memset(x2e[0:64, 83 * WB :], 0.0)
            nc.vector.memset(x2vz[0:64, 2:83, 0:1], 0.0)
            nc.vector.memset(x2vz[0:64, 2:83, 161:162], 0.0)
            nc.vector.memset(x2e[64:128, 0:WB], 0.0)
            nc.vector.memset(x2e[64:128, 82 * WB :], 0.0)
            nc.vector.memset(x2vz[64:128, 1:82, 0:1], 0.0)
            nc.vector.memset(x2vz[64:128, 1:82, 161:162], 0.0)
            nc.vector.memset(ts8[:, FLATG - 163 :], 0.0)
            # small zeros row for zeroing the DRAM depth pad
            zrow = singles.tile([1, 4096], F16)
            nc.vector.memset(zrow[:], 0.0)

            nc.gpsimd.dma_start(out=wt[0:64], in_=wt_d[:])
            nc.gpsimd.dma_start(out=wt[64:128], in_=wt_d[:])
            nc.sync.dma_start(
                out=b2[:], in_=b_d.rearrange("(p one) -> p one", one=1)
            )

            # contiguous fp32->fp16 x load into x2o's storage (staging),
            # then DVE scatter into the padded layout.  320-byte strided
            # DMA descriptors were the V1 prelude bottleneck.
            XCON = 81 * W  # 12960
            for r4 in range(0, 81, 21):
                rr = min(21, 81 - r4)
                nc.gpsimd.dma_start(
                    out=x2o[0:64, r4 * W : (r4 + rr) * W],
                    in_=x_d[:, r4 : r4 + rr, :].rearrange(
                        "c r w -> c (r w)"
                    ),
                )
                nc.gpsimd.dma_start(
                    out=x2o[64:128, r4 * W : (r4 + rr) * W],
                    in_=x_d[:, 79 + r4 : 79 + r4 + rr, :].rearrange(
                        "c r w -> c (r w)"
                    ),
                )
            x2v = x2e.rearrange("p (r w) -> p r w", r=NROWG, w=WB)
            stv = x2o[:, 0:XCON].rearrange("p (r w) -> p r w", r=81, w=W)
            for r4 in range(0, 81, 21):
                rr = min(21, 81 - r4)
                nc.vector.tensor_copy(
                    x2v[0:64, 2 + r4 : 2 + r4 + rr, 1:161],
                    stv[0:64, r4 : r4 + rr],
                )
                nc.vector.tensor_copy(
                    x2v[64:128, 1 + r4 : 1 + r4 + rr, 1:161],
                    stv[64:128, r4 : r4 + rr],
                )
            # odd-parity copy: x2o[:, j] = x2e[:, j+1], in 4 chunks
            # (overwrites the staging; WAR ordering handled by Tile)
            ch4 = (FLATG - 2) // 4 + 1
            for c4 in range(4):
                a4 = c4 * ch4
                b4 = min(FLATG - 2, a4 + ch4)
                nc.sync.dma_start(
                    out=x2o[:, a4:b4], in_=x2e[:, a4 + 1 : b4 + 1]
                )

            # padded fp16 depth in DRAM (bounce): contiguous flat reads for
            # the shifted dp rows and the replicated d0 rows.
            dpad = dramp.tile([2, FLATG], F16)
            zi = 0
            for h in range(2):
                done_z = 0
                while done_z < FLATG:
                    nz = min(4096, FLATG - done_z)
                    eng = nc.scalar if zi % 2 == 0 else nc.sync
                    eng.dma_start(
                        out=dpad[h : h + 1, done_z : done_z + nz],
                        in_=zrow[0:1, 0:nz],
                    )
                    zi += 1
                    done_z += nz
            dpv = dpad.rearrange("t (r w) -> t r w", r=NROWG, w=WB)
            nc.gpsimd.dma_start(
                out=dpv[0:1, 2:83, 1:161],
                in_=_lift(d_d[0:81, :]),
            )
            nc.gpsimd.dma_start(
                out=dpv[1:2, 1:82, 1:161],
                in_=_lift(d_d[79:160, :]),
            )
            dpad_f = dpad[:]
            # dp rows: ts8[m+4h, g] = D_h[g + off] -- contiguous reads
            for m, (dh, dw, off) in enumerate(MAPS):
                for h in range(2):
                    row = m + 4 * h
                    a = max(0, -off)
                    bnd = min(FLATG, FLATG - off)
                    src = bass.AP(
                        tensor=dpad_f.tensor,
                        offset=dpad_f.offset + h * FLATG + a + off,
                        ap=[[0, 1], [1, bnd - a]],
                    )
                    nc.scalar.dma_start(out=ts8[row : row + 1, a:bnd], in_=src)

            if sel is not None:
                nc.gpsimd.dma_start(out=sel[:], in_=sel_d[:])

            # ------------- compact sim, segmented: t -= d0, |t|, exp(-t)
            s8d = dramp.tile([8, FLATG], F16)
            for s in range(NCW):
                d08 = d08p.tile([8, DWIN], F16, tag="d08")
                w0 = DWIN * s
                for h in range(2):
                    src = bass.AP(
                        tensor=dpad_f.tensor,
                        offset=dpad_f.offset + h * FLATG + w0,
                        ap=[[0, 4], [1, DWIN]],
                    )
                    nc.scalar.dma_start(
                        out=d08[4 * h : 4 * h + 4, :], in_=src
                    )
                nc.vector.tensor_sub(
                    ts8[:, w0 : w0 + DWIN],
                    ts8[:, w0 : w0 + DWIN],
                    d08[:, :],
                )
                nc.scalar.activation(
                    out=ts8[:, w0 : w0 + DWIN],
                    in_=ts8[:, w0 : w0 + DWIN],
                    func=Abs,
                )
                nc.scalar.activation(
                    out=ts8[:, w0 : w0 + DWIN],
                    in_=ts8[:, w0 : w0 + DWIN],
                    func=Exp,
                    scale=-1.0,
                )
                nc.scalar.dma_start(
                    out=s8d[:, w0 : w0 + DWIN], in_=ts8[:, w0 : w0 + DWIN]
                )

            # ---------------- main loop
            for s in range(NSEG):
                qs = Q0 + s * SEGQ
                winbase = GUARD + qs - HALO       # even
                sims = []
                for m, (dh, dw, off) in enumerate(MAPS):
                    sim_m = simp.tile([128, WIN], F16, tag="sim")
                    sims.append(sim_m)
                    if m in PE_MAPS:
                        i = PE_MAPS.index(m)
                        done = 0
                        while done < WIN:
                            nb = min(1024, WIN - done)
                            ps = bpsum.tile([128, 1024], F32, tag="bps")
                            o2 = 0
                            while o2 < nb:
                                nn2 = min(512, nb - o2)
                                nc.tensor.matmul(
                                    ps[:, o2 : o2 + nn2],
                                    sel[:, i, :],
                                    ts8[
                                        :,
                                        winbase + done + o2 :
                                        winbase + done + o2 + nn2,
                                    ],
                                    start=True,
                                    stop=True,
                                )
                                o2 += nn2
                            nc.scalar.activation(
                                out=sim_m[:, done : done + nb],
                                in_=ps[:, 0:nb],
                                func=Ident,
                                scale=1.0,
                            )
                            done += nb
                    else:
                        for h in range(2):
                            row = m + 4 * h
                            base = s8d[:]
                            src = bass.AP(
                                tensor=base.tensor,
                                offset=base.offset + row * FLATG + winbase,
                                ap=[[0, 64], [1, WIN]],
                            )
                            nc.gpsimd.dma_start(
                                out=sim_m[64 * h : 64 * h + 64, :], in_=src
                            )

                prods = []
                for m in range(4):
                    pr = prodp.tile([128, WIN], F16, tag="prod")
                    prods.append(pr)
                    nc.vector.tensor_mul(
                        pr[:], x2e[:, winbase : winbase + WIN], sims[m][:]
                    )

                stg = stgp.tile([128, SEGROWS * W], F16, tag="stg")
                for j in range(NCH):
                    q = qs + j * CHW
                    so = j * CHW + HALO           # even
                    # width padded to 1024 (2 banks) so each partition's
                    # row is bank-aligned; only [:, :CHW] is used.
                    psum = cpsum.tile([128, 1024], F32, tag="cps")
                    xms = []
                    for m, (dh, dw, off) in enumerate(MAPS):
                        xm = xmp.tile([128, CHW], F16, tag="xm")
                        xms.append(xm)
                        if off % 2:
                            xsrc = x2o[
                                :,
                                GUARD + q + off - 1 :
                                GUARD + q + off - 1 + CHW,
                            ]
                        else:
                            xsrc = x2e[
                                :, GUARD + q + off : GUARD + q + off + CHW
                            ]
                        nc.vector.tensor_mul(
                            xm[:], xsrc, sims[m][:, so : so + CHW]
                        )
                    # taps as (wt-idx, rhs-fn); top/bot matmuls adjacent so
                    # the PE can run the two 64x64 quadrants concurrently.
                    o2 = 0
                    for si_, nn2 in enumerate(SUBS):
                        taps = []
                        for m, (dh, dw, off) in enumerate(MAPS):
                            po = so - off
                            taps.append(
                                (_tapidx(dh, dw), xms[m], o2)
                            )
                            taps.append(
                                (_tapidx(-dh, -dw), prods[m], po + o2)
                            )
                        taps.append((_tapidx(0, 0), x2e, GUARD + q + o2))
                        for ti, (widx, rsrc, roff) in enumerate(taps):
                            for half in range(2):
                                pl, ph = 64 * half, 64 * half + 64
                                nc.tensor.matmul(
                                    psum[pl:ph, o2 : o2 + nn2],
                                    wt[pl:ph, widx, :],
                                    rsrc[pl:ph, roff : roff + nn2],
                                    start=(ti == 0),
                                    stop=(ti == len(taps) - 1),
                                    skip_group_check=True,
                                )
                        o2 += nn2
                    # CHW = 810 = 5 padded rows; strip the pad columns in
                    # the evacuation (strided psum read, contiguous out)
                    pv = psum.rearrange("p (r w) -> p r w", r=1024 // 2, w=2)
                    nc.scalar.activation(
                        out=stg[
                            :, j * 5 * W : (j + 1) * 5 * W
                        ].rearrange("p (r w) -> p r w", r=5, w=W),
                        in_=bass.AP(
                            tensor=psum[:].tensor,
                            offset=psum[:].offset + 1,
                            ap=[list(psum[:].ap[0]), [WB, 5], [1, W]],
                        ),
                        func=Ident,
                        bias=b2[:],
                        scale=1.0,
                    )

                r0 = SEGROWS * s
                nc.gpsimd.dma_start(
                    out=out_d[:, r0 : r0 + SEGROWS, :].rearrange(
                        "c r w -> c (r w)"
                    ),
                    in_=stg[0:64, :],
                )
                nc.gpsimd.dma_start(
                    out=out_d[:, 80 + r0 : 80 + r0 + SEGROWS, :].rearrange(
                        "c r w -> c (r w)"
                    ),
                    in_=stg[64:128, :],
                )

    return nc


@functools.lru_cache(maxsize=1)
def _get_program():
    return _build_program()


def make_in_maps(x, depth, weights, bias):
    wt = np.ascontiguousarray(
        weights.reshape(O, C, KK).transpose(1, 2, 0)
    ).astype(np.float32)
    b2 = np.concatenate([bias, bias]).astype(np.float32)
    selm = np.zeros((8, max(1, len(PE_MAPS)), 128), np.float32)
    for i, m in enumerate(PE_MAPS):
        selm[m, i, 0:64] = 1.0
        selm[m + 4, i, 64:128] = 1.0
    base = {"wt": wt, "bias2": b2}
    if PE_MAPS:
        base["selm"] = selm
    return [
        {
            "x": np.ascontiguousarray(x[i]).astype(np.float32),
            "depth": np.ascontiguousarray(depth[i, 0]).astype(np.float32),
            **base,
        }
        for i in range(x.shape[0])
    ]


def kernel(x, depth, weights, bias):
    nc = _get_program()
    if not nc.is_finalized():
        nc.finalize()
    in_maps = make_in_maps(x, depth, weights, bias)
    res = run_bass_kernel_spmd(nc, in_maps, list(range(NCORES)))
    out = np.stack([np.asarray(res.results[i]["out"]) for i in range(NCORES)])
    return out.astype(np.float32)
